# revision 20
# baseline (speedup 1.0000x reference)
"""Multi-head self-attention (B=2, S=2048, d_model=1024, H=16, RoPE, causal)
on 8 Trainium2 NeuronCores, tensor-parallel over heads (2 heads/core).

v3, restructured around the v2 trace:
  - core-launch skew (~20-45us) was being absorbed by the FIRST real
    AllToAll (25.8us vs 6-9us for the rest) -> a tiny warm-up AllToAll
    posted at t~10us pays the skew + ncfw setup while the PE still has
    a deep well of independent work.
  - DMA rings are strict FIFO, so traffic classes are segregated:
      sync (SP) ring   : RoPE partner-swap strips + outproj rhs/outT
                         (small, latency-critical; never behind MB loads)
      scalar (ACT) ring: weights + x blocks, issued just-in-time 2 deep
      gpsimd ring      : tables, Wo, re-shard scatters, collectives
    (v2 put x-prefetch on the sync ring, which head-of-line blocked the
    RoPE strips for ~10us at a time.)
  - outproj rhs loads that wait on an AllToAll semaphore are emitted
    only at points where that A2A is already complete, or where nothing
    latency-critical sits behind them on the sync ring.
  - final 3 outproj chunks: rhs preloaded before attention(1,3), the
    matmuls run inside the last A2A window; batch-1 re-shard tapers
    (0,1),(2),(3) so the last A2A is half-size.
  - blocks (0,0)+(0,1) are emitted zipper-interleaved so each RoPE
    ladder hides under the other block's projection matmuls.

Per-core layout (core c owns heads 2c, 2c+1):
  - host pre-transposes x -> xT [1024, 4096] bf16; per-core transposed
    weight slices; Wq/Wk rows de-interleaved per head ([evens, odds]) so
    RoPE's partner swap is 4 partition-strip copies (SBUF->SBUF DMA).
  - q/k produced transposed ([dim, tok]); V re-transposed to [tok, dim]
    via PE transposes with an appended ones-block so the P@V matmul also
    yields the softmax denominator.
  - scores computed transposed per 128-wide k-tile with both heads packed
    in the PE via tile_position; exp on the scalar engine straight out of
    PSUM with the 1/sqrt(64) scale folded in; causal masking via a
    post-exp {0,1} multiply on the diagonal 128x128 block only.
  - attention outputs re-sharded head-parallel -> token-parallel with one
    AllToAll per (batch, chunk); each core then computes the FULL d_model
    output projection for its token slices using a full copy of Wo. Host
    reassembles by token slice.
"""

from collections import deque

import ml_dtypes
import numpy as np

import concourse.bass as bass
import concourse.mybir as mybir
import concourse.tile as tile
from concourse import bacc
from concourse.bass_utils import run_bass_kernel_spmd

P = 128
B, S, D = 2, 2048, 1024
T = B * S          # 4096 flattened tokens
H = 16
DH = 64            # head dim
NC = 8             # cores
HPC = H // NC      # heads per core = 2
DPC = HPC * DH     # dims per core = 128
KT = D // P        # 8 contraction tiles for d_model
TB = 512           # token block for projections
G = 512            # attention q-group width
NG = S // G        # 4 groups per batch
ROPE_THETA = 10000.0

# AllToAll chunking (units of 512-token attention groups). Batch 1
# tapers so the final A2A is small and overlappable.
CHA = {0: [(0, 1), (2, 3)], 1: [(0, 1), (2,), (3,)]}

F32 = mybir.dt.float32
BF = mybir.dt.bfloat16

_CACHE = {}


def _build():
    nc = bacc.Bacc(None, target_bir_lowering=False)

    xT = nc.dram_tensor("xT", [D, T], BF, kind="ExternalInput")
    wq = nc.dram_tensor("wq", [D, DPC], BF, kind="ExternalInput")
    wk = nc.dram_tensor("wk", [D, DPC], BF, kind="ExternalInput")
    wv = nc.dram_tensor("wv", [D, DPC], BF, kind="ExternalInput")
    wo = nc.dram_tensor("wo", [D, D], BF, kind="ExternalInput")
    cosb = nc.dram_tensor("cosb", [P, S], BF, kind="ExternalInput")
    sinb = nc.dram_tensor("sinb", [P, S], BF, kind="ExternalInput")
    maskb = nc.dram_tensor("maskb", [P, P], BF, kind="ExternalInput")
    iden = nc.dram_tensor("iden", [P, P], BF, kind="ExternalInput")
    outT = nc.dram_tensor("outT", [T // NC, D], BF, kind="ExternalOutput")

    with tile.TileContext(nc) as tc:
        with (
            tc.tile_pool(name="cst", bufs=1) as cst,
            tc.tile_pool(name="wpool", bufs=1) as wpool,
            tc.tile_pool(name="xin", bufs=4) as xin,
            tc.tile_pool(name="qk", bufs=1) as qkpool,
            tc.tile_pool(name="tmp", bufs=3) as tmp,
            tc.tile_pool(name="pt", bufs=4) as ptpool,
            tc.tile_pool(name="att", bufs=2) as attp,
            tc.tile_pool(name="prj", bufs=1) as prj,
            tc.tile_pool(name="ps", bufs=1, space="PSUM") as ps,
            tc.tile_pool(name="dram", bufs=1, space="DRAM") as dram,
        ):
            # ---- ACT exp-table warm-up first: the table load (~1.4us)
            # must not stall the first attention group's exp.
            scr = cst.tile([1, 16], F32)
            nc.vector.memset(scr, 1.0)
            nc.scalar.activation(
                scr, scr, mybir.ActivationFunctionType.Exp, scale=0.0
            )

            # ---- collectives warm-up: a tiny AllToAll absorbs the
            # core-launch skew + ncfw first-collective setup (~20-45us)
            # while the PE still has a deep well of independent work.
            wu_in = dram.tile([NC, 32], BF, name="wua")
            wu_out = dram.tile([NC, 32], BF, name="wub")
            wu_sb = cst.tile([NC, 32], BF)
            nc.gpsimd.memset(wu_sb, 0.0)
            nc.gpsimd.dma_start(wu_in[:, :], wu_sb)
            nc.gpsimd.collective_compute(
                "AllToAll",
                mybir.AluOpType.bypass,
                replica_groups=[list(range(NC))],
                ins=[wu_in[:, :]],
                outs=[wu_out[:, :]],
            )

            # ---- constants + block-(0,0) loads, segregated by ring ----
            # contraction dims are grouped (p ro): partition p holds the
            # KT consecutive rows 8p..8p+8 of each [D, *] operand, so
            # weight / rhs DMAs move 2KB-contiguous chunks per partition
            # instead of 256B strips. Both matmul operands regroup
            # identically, so the contraction result is unchanged.
            xTr = xT.rearrange("(p ro) t -> p ro t", p=P)
            ws = {}
            for name, w in (("q", wq), ("k", wk), ("v", wv)):
                ws[name] = wpool.tile([P, KT, DPC], BF, name=f"w{name}")
            ws["o"] = wpool.tile([P, KT, D], BF, name="wo")  # full Wo^T
            cos_t = cst.tile([P, S], BF)
            sin_t = cst.tile([P, S], BF)
            mk01 = cst.tile([P, P], BF)
            idn = cst.tile([P, P], BF)

            K2 = KT // 2
            xbs = {}

            def issue_x(b, j, eng):
                xbs[(b, j)] = xin.tile(
                    [P, KT, TB], BF, name="xb", tag="xb", bufs=4
                )
                eng.dma_start(
                    xbs[(b, j)], xTr[:, :, S * b + j * TB : S * b + (j + 1) * TB]
                )

            xbs[(0, 0)] = xin.tile([P, KT, TB], BF, name="xb", tag="xb", bufs=4)
            wqr = wq.rearrange("(p ro) m -> p ro m", p=P)
            nc.sync.dma_start(ws["q"][:, 0:K2], wqr[:, 0:K2])
            nc.scalar.dma_start(ws["k"], wk.rearrange("(p ro) m -> p ro m", p=P))
            nc.sync.dma_start(xbs[(0, 0)][:, 0:K2], xTr[:, 0:K2, 0:TB])
            nc.scalar.dma_start(idn, iden[:, :])
            nc.sync.dma_start(ws["q"][:, K2:], wqr[:, K2:])
            nc.sync.dma_start(xbs[(0, 0)][:, K2:], xTr[:, K2:, 0:TB])
            nc.scalar.dma_start(ws["v"], wv.rearrange("(p ro) m -> p ro m", p=P))
            nc.gpsimd.dma_start(cos_t[:, 0 : S // 2], cosb[:, 0 : S // 2])
            nc.gpsimd.dma_start(sin_t[:, 0 : S // 2], sinb[:, 0 : S // 2])
            nc.gpsimd.dma_start(mk01, maskb[:, :])
            issue_x(0, 1, nc.scalar)
            issue_x(0, 2, nc.scalar)
            nc.gpsimd.dma_start(cos_t[:, S // 2 :], cosb[:, S // 2 :])
            nc.gpsimd.dma_start(sin_t[:, S // 2 :], sinb[:, S // 2 :])

            gmap = {}
            for b in range(B):
                for a, grps in enumerate(CHA[b]):
                    for gg, g in enumerate(grps):
                        gmap[(b, g)] = (a, gg)
            ag_in = [
                [
                    dram.tile(
                        [NC * DPC, len(gr) * G // NC], BF, name=f"a2in{b}_{a}"
                    )
                    for a, gr in enumerate(CHA[b])
                ]
                for b in range(B)
            ]
            ag_out = [
                [
                    dram.tile(
                        [NC * DPC, len(gr) * G // NC], BF, name=f"a2out{b}_{a}"
                    )
                    for a, gr in enumerate(CHA[b])
                ]
                for b in range(B)
            ]
            # outT column base per (b, chunk)
            obase = {}
            ob0 = 0
            for b in range(B):
                for a, gr in enumerate(CHA[b]):
                    obase[(b, a)] = ob0
                    ob0 += len(gr) * G // NC

            qTs, kTs, vxs = {}, {}, {}

            def units_block(b, j):
                """Generator: project x block j of batch b -> qT/kT (roped)
                and vx tiles; yields between PE units for interleaving."""
                js = slice(j * TB, (j + 1) * TB)
                if j == 0:
                    qTs[b] = qkpool.tile([P, S], BF, name="qT", tag=f"qT{b}")
                    kTs[b] = qkpool.tile([P, S], BF, name="kT", tag=f"kT{b}")
                    vxs[b] = [
                        qkpool.tile([P, S // P, P], BF, name=f"vx{h}", tag=f"vx{b}_{h}")
                        for h in range(HPC)
                    ]
                    # ones-block FIRST so the P@V denominator lands on
                    # partitions 0:64 (reciprocal_approx_fast mishandles
                    # base-partition-shifted inputs)
                    for h in range(HPC):
                        nc.gpsimd.memset(vxs[b][h][:, :, 0:DH], 1.0)
                xb = xbs[(b, j)]
                yield
                for name in ("q", "k", "v"):
                    pp = ps.tile([P, TB], F32, name="pp", tag="pp", bufs=2)
                    for k in range(KT):
                        nc.tensor.matmul(
                            pp,
                            ws[name][:, k],
                            xb[:, k],
                            start=(k == 0),
                            stop=(k == KT - 1),
                        )
                        yield
                    if name == "v":
                        vr = tmp.tile([P, TB], BF, name="vr", tag="vr")
                        nc.vector.tensor_copy(vr, pp)
                        yield
                        vtp = ps.tile(
                            [P, TB // P, P], BF, name="vtp", tag="pp", bufs=2
                        )
                        for t4 in range(TB // P):
                            nc.tensor.transpose(
                                vtp[:, t4], vr[:, t4 * P : (t4 + 1) * P], idn
                            )
                            yield
                        for h in range(HPC):
                            nc.vector.tensor_copy(
                                vxs[b][h][:, 4 * j : 4 * j + 4, DH:P],
                                vtp[:, :, DH * h : DH * (h + 1)],
                            )
                        yield
                    else:
                        raw = tmp.tile([P, TB], BF, name="raw", tag=f"{name}raw")
                        nc.scalar.copy(raw, pp)
                        gsw = tmp.tile([P, TB], BF, name="gsw", tag=f"{name}g")
                        # partner-swap strips split across the sync and
                        # gpsimd rings (each ring serializes transfers, so
                        # 2+2 halves the ~2.6us all-on-one-ring latency)
                        for e, (s0, s1) in zip(
                            (nc.sync, nc.gpsimd, nc.sync, nc.gpsimd),
                            ((0, 32), (32, 0), (64, 96), (96, 64)),
                        ):
                            e.dma_start(gsw[s0 : s0 + 32], raw[s1 : s1 + 32])
                        dstT = qTs[b] if name == "q" else kTs[b]
                        t1 = tmp.tile([P, TB], BF, name="t1", tag="t1")
                        nc.vector.tensor_tensor(
                            t1, raw, cos_t[:, js], mybir.AluOpType.mult
                        )
                        yield
                        t2 = tmp.tile([P, TB], BF, name="t2", tag="t2")
                        nc.vector.tensor_tensor(
                            t2, gsw, sin_t[:, js], mybir.AluOpType.mult
                        )
                        nc.vector.tensor_tensor(
                            dstT[:, js], t1, t2, mybir.AluOpType.add
                        )
                        yield

            def load_rhs(b, chunks):
                """Issue the re-shard result loads for outproj chunks."""
                ww = [len(CHA[b][a]) * G // NC for a in chunks]
                w = sum(ww)
                rhs = prj.tile(
                    [P, KT, w], BF, name="rhs", tag=f"rhs{b}_{chunks[0]}"
                )
                off = 0
                for a, wa in zip(chunks, ww):
                    nc.sync.dma_start(
                        rhs[:, :, off : off + wa],
                        ag_out[b][a].rearrange("(p ro) t -> p ro t", p=P),
                    )
                    off += wa
                return rhs, w

            def units_outproj(b, chunks, pre=None):
                """Generator: full-d_model output projection of my token
                slices for the given (consecutive) chunks of batch b."""
                rhs, w = pre if pre is not None else load_rhs(b, chunks)
                yield
                # transposed form: stationary = my tokens, moving = Wo
                # rows -> 16 wide 512-free matmuls instead of 64 narrow ones
                obt = prj.tile(
                    [P, 2, D // 2], BF, name="obt", tag=f"obt{b}_{chunks[0]}"
                )
                for m in range(2):
                    po = ps.tile([P, D // 2], F32, name="po", tag="pp", bufs=2)
                    for k in range(KT):
                        nc.tensor.matmul(
                            po[0:w],
                            rhs[:, k],
                            ws["o"][:, k, m * (D // 2) : (m + 1) * (D // 2)],
                            start=(k == 0),
                            stop=(k == KT - 1),
                        )
                        if k % 2 == 1:
                            yield
                    nc.vector.tensor_copy(obt[0:w, m], po[0:w])
                cs = slice(obase[(b, chunks[0])], obase[(b, chunks[0])] + w)
                nc.sync.dma_start(outT[cs, :], obt[0:w])
                yield

            fill = deque()

            def pull(n):
                while n > 0 and fill:
                    try:
                        next(fill[0])
                        n -= 1
                    except StopIteration:
                        fill.popleft()

            def exhaust_fill():
                while fill:
                    for _ in fill.popleft():
                        pass

            def attention_qgroup(b, g):
                """Causal attention for q-cols [g*G, (g+1)*G) of batch b."""
                qT, kT, vx = qTs[b], kTs[b], vxs[b]
                oa = ps.tile([P, HPC, G], F32, name="oa", tag="oa", bufs=1)
                n_t = (g + 1) * G // P  # valid k-tiles
                pend = None  # software pipeline: P@V lags scores by one t

                def p_at_v(t, c0, pT):
                    for h in range(HPC):
                        nc.tensor.matmul(
                            oa[:, h, c0:],
                            vx[h][:, t],
                            pT[:, h, c0:],
                            start=(t == 0),
                            stop=(t == n_t - 1),
                            skip_group_check=True,
                        )

                for t in range(n_t):
                    c0 = max(0, t * P - g * G)
                    sc = ps.tile([P, HPC, G], F32, name="sc", tag="sc", bufs=2)
                    for h in range(HPC):
                        hs = slice(DH * h, DH * (h + 1))
                        nc.tensor.matmul(
                            sc[:, h, c0:],
                            kT[hs, t * P : (t + 1) * P],
                            qT[hs, g * G + c0 : (g + 1) * G],
                            start=True,
                            stop=True,
                            tile_position=(DH * h, 0),
                        )
                    pT = ptpool.tile([P, HPC, G], BF, name="pT", tag="pT")
                    nc.scalar.activation(
                        pT[:, :, c0:],
                        sc[:, :, c0:],
                        mybir.ActivationFunctionType.Exp,
                        scale=1.0 / np.sqrt(DH),
                    )
                    if t * P >= g * G:  # diagonal block: causal 0/1 mask
                        for h in range(HPC):
                            nc.vector.tensor_tensor(
                                pT[:, h, c0 : c0 + P],
                                pT[:, h, c0 : c0 + P],
                                mk01,
                                mybir.AluOpType.mult,
                            )
                    if pend is not None:
                        p_at_v(*pend)
                    pend = (t, c0, pT)
                    pull(3)
                p_at_v(*pend)
                a, gg = gmap[(b, g)]
                ng = len(CHA[b][a])
                w = ng * G // NC       # shard width (tokens per core)
                spg = NC // ng         # shard blocks this group spans
                agt = ag_in[b][a]
                rec = attp.tile([DH, HPC, G], F32, name="rec", tag="rec")
                at = attp.tile([DH, HPC, G], BF, name="at", tag="at")
                # per-head ladder so head 0's scatter issues while head 1
                # is still normalizing (shaves ~1us off the A2A post)
                for h in range(HPC):
                    nc.vector.reciprocal_approx_fast(rec[:, h], oa[0:DH, h])
                    nc.vector.tensor_tensor(
                        at[:, h], oa[DH:P, h], rec[:, h], mybir.AluOpType.mult
                    )
                    # scatter my [64, 512] per-head slab into shard blocks:
                    # dst row = (spg*gg + s)*128 + 64*h + p, col = token
                    dst = bass.AP(
                        tensor=agt[:, :].tensor,
                        offset=gg * spg * P * w + DH * h * w,
                        ap=[[w, DH], [P * w, spg], [1, w]],
                    )
                    nc.gpsimd.dma_start(dst, at[:, h])
                if gg == ng - 1:
                    nc.gpsimd.collective_compute(
                        "AllToAll",
                        mybir.AluOpType.bypass,
                        replica_groups=[list(range(NC))],
                        ins=[ag_in[b][a][:, :]],
                        outs=[ag_out[b][a][:, :]],
                    )

            def exhaust(gen):
                for _ in gen:
                    pass

            def zipn(*gens):
                """Zipper-interleave unit generators."""
                live = list(gens)
                while live:
                    for g in list(live):
                        try:
                            next(g)
                        except StopIteration:
                            live.remove(g)

            # ---- fused emission pipeline ----
            # Batches' attention groups interleave so one batch's group-end
            # ladder (recip/at-mult/A2A) hides under the other's compute,
            # and the A2As spread across the kernel instead of bunching.
            # x loads are issued 2 blocks ahead on the scalar ring.
            # two-block zipper start: each RoPE ladder hides under the
            # other block's projection matmuls. NOT three blocks: block
            # (0,2)'s first matmul would sit 3rd in the strict-FIFO PE
            # queue waiting on its x DMA, and that nondeterministic stall
            # desyncs the 8 cores (every A2A then pays 10-15us of skew).
            zipn(units_block(0, 0), units_block(0, 1))
            issue_x(1, 0, nc.scalar)
            fill.append(units_block(0, 2))
            attention_qgroup(0, 0)
            exhaust_fill()
            # full Wo (2MB, needed ~90us in): late so it doesn't steal
            # HBM bandwidth from the startup-critical loads; on the
            # scalar ring so it never sits ahead of re-shard scatters or
            # RoPE strips on the gpsimd ring
            nc.scalar.dma_start(ws["o"], wo.rearrange("(p ro) m -> p ro m", p=P))
            issue_x(1, 1, nc.scalar)
            fill.append(units_block(1, 0))
            attention_qgroup(0, 1)  # A2A(0,0) fires at its end
            exhaust_fill()
            issue_x(0, 3, nc.scalar)
            fill.append(units_block(1, 1))
            attention_qgroup(1, 0)
            exhaust_fill()
            pre00 = load_rhs(0, [0])  # A2A(0,0) completed during att(1,0)
            issue_x(1, 2, nc.scalar)
            fill.append(units_block(0, 3))
            attention_qgroup(0, 2)
            exhaust_fill()
            issue_x(1, 3, nc.scalar)
            fill.append(units_block(1, 2))
            attention_qgroup(1, 1)  # A2A(1,0) fires
            exhaust_fill()
            fill.append(units_block(1, 3))
            attention_qgroup(0, 3)  # A2A(0,1) fires
            exhaust_fill()
            # preload outproj rhs right after their A2As are safely done
            # (waits, if any, only delay other rhs/outT on the ring)
            pre10 = load_rhs(1, [0])
            attention_qgroup(1, 2)  # A2A(1,1) fires (512 tokens)
            exhaust_fill()
            pre01 = load_rhs(0, [1])
            attention_qgroup(1, 3)  # A2A(1,2) fires (512 tokens, small)
            # must be emitted AFTER the A2A posts (program-order WAR)
            pre1x = load_rhs(1, [1, 2])
            exhaust_fill()
            # ALL outproj passes are held back here (~14us of preloaded
            # PE work): they fill the final A2A window AND absorb core
            # skew, since this is the only cross-core wait left. The two
            # 64-token chunks merge into one pass after the last A2A.
            exhaust(units_outproj(0, [0], pre=pre00))
            exhaust(units_outproj(1, [0], pre=pre10))
            exhaust(units_outproj(0, [1], pre=pre01))
            exhaust(units_outproj(1, [1, 2], pre=pre1x))

    nc.compile()
    return nc


def _host_inputs(x, token_positions, Wq, Wk, Wv, Wo):
    xT = np.ascontiguousarray(x.reshape(T, D).T).astype(ml_dtypes.bfloat16)  # [D, T]

    # de-interleave perm within each 64-dim head: [evens, odds]
    perm = np.concatenate(
        [64 * h + np.r_[np.arange(0, 64, 2), np.arange(1, 64, 2)] for h in range(HPC)]
    )

    pos = token_positions.astype(np.float64)  # [S]
    inv_freq = ROPE_THETA ** (-np.arange(0, DH, 2, dtype=np.float64) / DH)  # [32]
    ang = pos[:, None] * inv_freq[None, :]  # [S, 32]
    cos = np.cos(ang).T.astype(np.float32)  # [32, S]
    sin = np.sin(ang).T.astype(np.float32)
    cosb = np.concatenate([cos, cos, cos, cos], axis=0).astype(ml_dtypes.bfloat16)
    sinb = np.concatenate([-sin, sin, -sin, sin], axis=0).astype(ml_dtypes.bfloat16)

    maskb = np.triu(np.ones((P, P), dtype=np.float32)).astype(ml_dtypes.bfloat16)
    iden = np.eye(P, dtype=np.float32).astype(ml_dtypes.bfloat16)

    woT = np.ascontiguousarray(Wo.T).astype(ml_dtypes.bfloat16)  # [in, out]
    in_maps = []
    for c in range(NC):
        rs = slice(DPC * c, DPC * (c + 1))
        in_maps.append(
            {
                "xT": xT,
                "wq": np.ascontiguousarray(Wq[rs][perm].T).astype(ml_dtypes.bfloat16),
                "wk": np.ascontiguousarray(Wk[rs][perm].T).astype(ml_dtypes.bfloat16),
                "wv": np.ascontiguousarray(Wv[rs].T).astype(ml_dtypes.bfloat16),
                "wo": woT,
                "cosb": cosb,
                "sinb": sinb,
                "maskb": maskb,
                "iden": iden,
            }
        )
    return in_maps


def kernel(x, token_positions, Wq, Wk, Wv, Wo, _trace=False, _result=[None]):
    x = np.asarray(x, dtype=np.float32)
    token_positions = np.asarray(token_positions)
    Wq, Wk, Wv, Wo = (np.asarray(w, dtype=np.float32) for w in (Wq, Wk, Wv, Wo))

    if "nc" not in _CACHE:
        _CACHE["nc"] = _build()
    nc = _CACHE["nc"]

    in_maps = _host_inputs(x, token_positions, Wq, Wk, Wv, Wo)
    res = run_bass_kernel_spmd(nc, in_maps, core_ids=list(range(NC)), trace=_trace)
    _result[0] = res
    out = np.empty((B, S, D), dtype=np.float32)
    for c in range(NC):
        r = np.asarray(res.results[c]["outT"], dtype=np.float32)  # [S//NC*B, D]
        ob0 = 0
        for b in range(B):
            for grps in CHA[b]:
                w = len(grps) * G // NC
                ts = grps[0] * G + c * w
                out[b, ts : ts + w, :] = r[ob0 : ob0 + w, :]
                ob0 += w
    return out
